# revision 45
# baseline (speedup 1.0000x reference)
"""MoNet (2-layer GMMConv) Trainium2 Bass kernel -- self-contained.

Edge/graph parallelism, dst-sharded across 8 NeuronCores (each core owns a
contiguous destination-node range; its segment-sum is fully local, so no
cross-core reduction is needed):
 - Host (index preprocessing only): sorts edges by dst, shards by dst range,
   splits each 128-node tile's edges into src-lo/src-hi groups (so int16
   gather indices stay in range), packs per-core (P, TC) edge-slot arrays,
   the 16-wrapped int16 gather-index stream, and an fp8 one-hot stream of
   dst_local (one 128-wide one-hot per edge slot).
 - Device, per layer:
   (A) hp = x @ W.T bf16 gather table, built lo-half then hi-half into two
       DRAM tensors so edge batches overlap the build; two node tiles per
       PSUM bank; PSUM->SBUF copies alternate Activation/DVE.
   (W) per-edge mixture weights w[e,k] = exp(a_k + b.p + c.p^2) with
       p = tanh(pseudo @ pp_w.T + pp_b) (parameters pre-folded on host);
       weights stored pair-replicated (wk2[p,2c+d]=wk[p,c]) so the fold's
       broadcast operand keeps a packed [1,2] innermost dim and DVE runs
       its 2x perf mode.
   (B) per 16-column batch: dma_gather of hp[src] rows (4 SWDGE queues),
       DVE weight-fold to bf16 messages, one-hot streamed from host (fp8,
       via the Activation engine's DMA queue) or built on DVE (is_equal
       of pair-replicated dl vs paired iota, bf16, same [1,2] 2x trick)
       -- fraction tuned per layer to balance DMA vs DVE; PE matmul
       msg^T @ onehot (bf16 lhsT x fp8/bf16 rhs) accumulated per node
       tile in PSUM; bias added on the Activation engine; bf16
       accumulator; chunked output write.
 - The transposed (OUT, nodes) bf16 output is the next layer's input
   layout; host only transposes between the two launches.
"""

import sys
import numpy as np
import ml_dtypes

BFDT = ml_dtypes.bfloat16
FP8DT = ml_dtypes.float8_e4m3

for p in ("/opt/trn_rl_repo",):
    if p not in sys.path:
        sys.path.insert(0, p)

import concourse.bass as bass
import concourse.mybir as mybir
import concourse.tile as tile
from concourse import bacc
from concourse import bass_utils

F32 = mybir.dt.float32
BF16 = mybir.dt.bfloat16
I32 = mybir.dt.int32
I16 = mybir.dt.int16
FP8 = mybir.dt.float8e4

P = 128


def _half_rows(NT):
    return ((NT + 1) // 2) * P     # tile-aligned src half-table boundary


# ----------------------------------------------------------------------------
# Host-side metadata (index preprocessing / sharding)
# ----------------------------------------------------------------------------

def build_edge_metadata(src, dst, pseudo, n_nodes, n_cores, G=16):
    """Sort edges by dst, shard by destination-node range, and within each
    128-node tile split edges into src-lo/src-hi groups (half-table
    gather keeps int16 indices in range). Pack per-core (P, TC) arrays in
    [lo | hi] column order, the wrapped int16 gather index array (one
    16-row wrap per G-chunk batch), and the host-precomputed one-hot
    stream (bf16, per edge slot a 128-wide one-hot of dst_local%128)."""
    NT = -(-n_nodes // P)
    T = -(-NT // n_cores)
    NPAD = NT * P
    HALF = _half_rows(NT)

    order = np.argsort(dst, kind="stable")
    sdst = dst[order]
    ssrc = src[order]
    sps = pseudo[order]

    tile_g = sdst // P
    key = (ssrc >= HALF).astype(np.int64)
    gidx = np.where(key == 0, ssrc, ssrc - HALF)
    cnt = np.bincount(tile_g * 2 + key, minlength=n_cores * T * 2)
    cnt = cnt.reshape(-1, 2)
    C0 = max(1, int(-(-cnt[:, 0].max() // P)))
    C1 = max(1, int(-(-cnt[:, 1].max() // P)))
    TC = T * (C0 + C1)
    NSEG0 = T * C0

    starts = np.zeros(n_cores * T * 2, np.int64)
    grp = tile_g * 2 + key
    gcnt = cnt.reshape(-1)
    np.cumsum(gcnt[:-1], out=starts[1:])
    gorder = np.argsort(grp, kind="stable")
    rank = np.empty(len(grp), np.int64)
    rank[gorder] = np.arange(len(grp)) - starts[grp[gorder]]

    core = tile_g // T
    lt = tile_g % T
    ci = rank // P
    pp = rank % P
    col = np.where(key == 0,
                   lt * C0 + ci,
                   NSEG0 + lt * C1 + ci)

    src_t = np.zeros((n_cores, P, TC), np.int16)
    dl_t = np.full((n_cores, P, TC), -1.0, np.float32)
    ps_a = np.zeros((n_cores, P, TC), np.float32)
    ps_b = np.zeros((n_cores, P, TC), np.float32)
    src_t[core, pp, col] = gidx.astype(np.int16)
    dl_t[core, pp, col] = (sdst - tile_g * P).astype(np.float32)
    ps_a[core, pp, col] = sps[:, 0]
    ps_b[core, pp, col] = sps[:, 1]

    nb0 = -(-NSEG0 // G)
    nb1 = -(-(TC - NSEG0) // G)
    SB = P * G // 16
    gat_w = np.zeros((n_cores, P, (nb0 + nb1) * SB), np.int16)

    def fill_batches(seg0, ncols, b0):
        for b in range(-(-ncols // G)):
            c0 = seg0 + b * G
            gn = min(G, seg0 + ncols - c0)
            flat = src_t[:, :, c0:c0 + gn].transpose(0, 2, 1).reshape(
                n_cores, gn * P)
            w = flat.reshape(n_cores, gn * P // 16, 16).transpose(0, 2, 1)
            gat_w[:, :, (b0 + b) * SB:(b0 + b) * SB + gn * P // 16] = \
                np.tile(w, (1, 8, 1))
    fill_batches(0, NSEG0, 0)
    fill_batches(NSEG0, TC - NSEG0, nb0)

    # host one-hot stream: (n_cores, P, TC*128) bf16; pads (dl=-1) -> zeros
    oh = (dl_t[..., None] == np.arange(P, dtype=np.float32)).astype(FP8DT)
    oh_w = np.ascontiguousarray(oh.reshape(n_cores, P, TC * P))

    return dict(gat_w=gat_w, dl_t=dl_t, oh_w=oh_w, ps_a=ps_a, ps_b=ps_b,
                dl_bf=np.ascontiguousarray(
                    np.repeat(dl_t, 2, axis=2).astype(BFDT)),
                psa_bf=np.ascontiguousarray(ps_a.astype(BFDT)),
                psb_bf=np.ascontiguousarray(ps_b.astype(BFDT)),
                T=T, C0=C0, C1=C1, NT=NT, G=G, nb0=nb0, nb1=nb1)


def pack_params(pp_w, pp_b, mu, inv_sigma):
    """Fold the Gaussian-mixture parameters into the quadratic form
    logw_k = a_k + sum_d b_kd p_d + c_kd p_d^2 (parameter-only algebra)."""
    K = mu.shape[0]
    is2 = inv_sigma.astype(np.float64) ** 2
    a = -0.5 * (is2 * mu.astype(np.float64) ** 2).sum(axis=1)
    b = is2 * mu
    c = -0.5 * is2
    par = np.zeros(32, np.float32)
    par[0] = pp_w[0, 0]; par[1] = pp_w[0, 1]
    par[2] = pp_w[1, 0]; par[3] = pp_w[1, 1]
    par[4] = pp_b[0]; par[5] = pp_b[1]
    par[6:6 + K] = a
    par[9:9 + K] = b[:, 0]
    par[12:12 + K] = b[:, 1]
    par[15:15 + K] = c[:, 0]
    par[18:18 + K] = c[:, 1]
    return par.reshape(1, 32)


# ----------------------------------------------------------------------------
# Device kernel builder (one GMMConv layer)
# ----------------------------------------------------------------------------

def build_layer_kernel(NT, T, C0, C1, IN_C, OUT, K=3, G=16, n_cores=8,
                       gat_bufs=6, oh_host_per8=6, copies_act=False):
    """One GMMConv layer. oh_host_per8: of every 8 edge batches, this many
    take their one-hot from the host stream (DMA); the rest build it on
    DVE. Balances the DVE and DMA resources."""
    ROWF = K * OUT
    ROWP = -(-ROWF // 128) * 128
    NPAD = NT * P
    TP = T * P
    HALF = _half_rows(NT)
    TC = T * (C0 + C1)
    NSEG0 = T * C0
    nb0 = -(-NSEG0 // G)
    nb1 = -(-(TC - NSEG0) // G)
    SB = P * G // 16
    NIDXCOLS = (nb0 + nb1) * SB

    nc = bacc.Bacc("TRN2", target_bir_lowering=False, debug=False,
                   num_devices=n_cores, num_swdge_queues=4)
    xT = nc.dram_tensor("xT", [IN_C, NPAD], BF16, kind="ExternalInput")
    wT = nc.dram_tensor("wT", [IN_C, ROWP], BF16, kind="ExternalInput")
    gat_d = nc.dram_tensor("gat_w", [P, NIDXCOLS], I16, kind="ExternalInput")
    dl_d = nc.dram_tensor("dl_t", [P, 2 * TC], BF16, kind="ExternalInput")
    oh_d = nc.dram_tensor("oh_w", [P, TC * P], FP8, kind="ExternalInput")
    psa_d = nc.dram_tensor("ps_a", [P, TC], BF16, kind="ExternalInput")
    psb_d = nc.dram_tensor("ps_b", [P, TC], BF16, kind="ExternalInput")
    par_d = nc.dram_tensor("par", [1, 32], F32, kind="ExternalInput")
    bias_d = nc.dram_tensor("bias", [OUT, 1], F32, kind="ExternalInput")
    hout_d = nc.dram_tensor("h_out", [OUT, TP], BF16, kind="ExternalOutput")
    hp_lo = nc.dram_tensor("hp_lo", [HALF, ROWP], BF16)
    hp_hi = nc.dram_tensor("hp_hi", [NPAD - HALF, ROWP], BF16)

    with tile.TileContext(nc) as tc:
        with (
            tc.tile_pool(name="const", bufs=1) as cst,
            tc.tile_pool(name="hps", bufs=4) as hpsp,
            tc.tile_pool(name="gat", bufs=gat_bufs) as gatp,
            tc.tile_pool(name="msg", bufs=5) as msgp,
            tc.tile_pool(name="oh", bufs=5) as ohp,
            tc.tile_pool(name="psB", bufs=6, space="PSUM") as psB,
        ):
            # ---- constants ----
            spar = cst.tile([P, 32], F32)
            nc.sync.dma_start(out=spar[:], in_=par_d[:].to_broadcast((P, 32)))
            sbias = cst.tile([OUT, 1], F32)
            nc.sync.dma_start(out=sbias[:], in_=bias_d[:])
            if oh_host_per8 < 8:
                iota_i = cst.tile([P, P], I32)
                nc.gpsimd.iota(iota_i[:], pattern=[[1, P]], base=0,
                               channel_multiplier=0)
                iota_f = cst.tile([P, P], BF16)
                nc.vector.tensor_copy(iota_f[:], iota_i[:])
            wTs = cst.tile([IN_C, ROWP], BF16)
            nc.sync.dma_start(out=wTs[:], in_=wT[:])

            gat_s = cst.tile([P, NIDXCOLS], I16)
            nc.sync.dma_start(out=gat_s[:], in_=gat_d[:])
            if oh_host_per8 < 8:
                dl_b = cst.tile([P, 2 * TC], BF16)
                nc.sync.dma_start(out=dl_b[:], in_=dl_d[:])

            # ---- phase A: hp table build (bf16, lo half then hi half) ----
            TPB = 512 // ROWP           # node tiles packed per PSUM bank
            WGRP = 4 * TPB              # hp row-tiles per write DMA
            BLKT = 2 * WGRP
            HT = HALF // P              # node tiles in the lo half

            cctr = [0]

            def table_pass(t_begin, t_end, dst, r_base, xblkp, psA,
                           act_only=False):
                for b in range(-(-(t_end - t_begin) // BLKT)):
                    t0 = t_begin + b * BLKT
                    tn = min(BLKT, t_end - t0)
                    xblk = xblkp.tile([IN_C, BLKT * P], BF16, tag="xblk")
                    nc.sync.dma_start(out=xblk[:, :tn * P],
                                      in_=xT[:, t0 * P:(t0 + tn) * P])
                    for i0 in range(0, tn, WGRP):
                        gn_w = min(WGRP, tn - i0)
                        hps = hpsp.tile([P, WGRP * ROWP], BF16, tag="hps")
                        for j0 in range(i0, i0 + gn_w, TPB):
                            jn = min(TPB, i0 + gn_w - j0)
                            pst = psA.tile([P, 512], F32)
                            for i in range(j0, j0 + jn):
                                nc.tensor.matmul(
                                    pst[:, (i - j0) * ROWP:
                                        (i - j0 + 1) * ROWP],
                                    lhsT=xblk[:, i * P:(i + 1) * P],
                                    rhs=wTs[:], start=True, stop=True)
                            if act_only or cctr[0] % 2 == 0:
                                nc.scalar.activation(
                                    hps[:, (j0 - i0) * ROWP:
                                        (j0 - i0 + jn) * ROWP],
                                    pst[:, :jn * ROWP],
                                    mybir.ActivationFunctionType.Copy)
                            else:
                                nc.vector.tensor_copy(
                                    hps[:, (j0 - i0) * ROWP:
                                        (j0 - i0 + jn) * ROWP],
                                    pst[:, :jn * ROWP])
                            cctr[0] += 1
                        r0 = (t0 + i0) * P - r_base
                        nc.sync.dma_start(
                            out=dst[r0:r0 + gn_w * P, :].rearrange(
                                "(g p) f -> p g f", p=P),
                            in_=hps[:, :gn_w * ROWP].rearrange(
                                "p (g f) -> p g f", f=ROWP))

            xblkp_cm = tc.tile_pool(name="xblk", bufs=3)
            xblkp = xblkp_cm.__enter__()
            psA_cm = tc.tile_pool(name="psA", bufs=2, space="PSUM")
            psA = psA_cm.__enter__()
            table_pass(0, HT, hp_lo, 0, xblkp, psA)

            # ---- phase W: edge weights (f32 math, bf16 result) ----
            def ts_mul(out, in0, j):
                nc.vector.tensor_scalar_mul(out, in0, spar[:, j:j + 1])

            wk = [cst.tile([P, TC], BF16, name=f"wk{k}", tag=f"w{k}")
                  for k in range(K)]
            wk2 = [cst.tile([P, 2 * TC], BF16, name=f"wk2{k}", tag=f"w2{k}")
                   for k in range(K)]
            TC2 = -(-TC // 2)
            with tc.tile_pool(name="wprep", bufs=1) as wpp, \
                 tc.tile_pool(name="wload", bufs=2) as wld:
              for h0 in range(2):
                cw0 = h0 * TC2
                cwn = min(TC2, TC - cw0)
                psa_s = wld.tile([P, TC2], BF16, tag="psa", name=f"psa{h0}")
                nc.sync.dma_start(out=psa_s[:, :cwn],
                                  in_=psa_d[:, cw0:cw0 + cwn])
                psb_s = wld.tile([P, TC2], BF16, tag="psb", name=f"psb{h0}")
                nc.sync.dma_start(out=psb_s[:, :cwn],
                                  in_=psb_d[:, cw0:cw0 + cwn])
                pa = wpp.tile([P, TC2], F32, tag="pa", name=f"pa{h0}")
                pb = wpp.tile([P, TC2], F32, tag="pb", name=f"pb{h0}")
                qa = wpp.tile([P, TC2], F32, tag="qa", name=f"qa{h0}")
                qb = wpp.tile([P, TC2], F32, tag="qb", name=f"qb{h0}")
                m1 = wpp.tile([P, TC2], F32, tag="m1", name=f"m1{h0}")
                m2 = wpp.tile([P, TC2], F32, tag="m2", name=f"m2{h0}")
                ts_mul(m1[:, :cwn], psa_s[:, :cwn], 0)
                ts_mul(m2[:, :cwn], psb_s[:, :cwn], 1)
                nc.vector.tensor_add(m1[:, :cwn], m1[:, :cwn], m2[:, :cwn])
                nc.scalar.activation(pa[:, :cwn], m1[:, :cwn],
                                     mybir.ActivationFunctionType.Tanh,
                                     bias=spar[:, 4:5])
                m3 = wpp.tile([P, TC2], F32, tag="m1", name=f"m3{h0}")
                m4 = wpp.tile([P, TC2], F32, tag="m2", name=f"m4{h0}")
                ts_mul(m3[:, :cwn], psa_s[:, :cwn], 2)
                ts_mul(m4[:, :cwn], psb_s[:, :cwn], 3)
                nc.vector.tensor_add(m3[:, :cwn], m3[:, :cwn], m4[:, :cwn])
                nc.scalar.activation(pb[:, :cwn], m3[:, :cwn],
                                     mybir.ActivationFunctionType.Tanh,
                                     bias=spar[:, 5:6])
                nc.scalar.activation(qa[:, :cwn], pa[:, :cwn],
                                     mybir.ActivationFunctionType.Square)
                nc.scalar.activation(qb[:, :cwn], pb[:, :cwn],
                                     mybir.ActivationFunctionType.Square)
                for k in range(K):
                    u1 = wpp.tile([P, TC2], F32, tag="m1", name=f"u1{h0}_{k}")
                    u2 = wpp.tile([P, TC2], F32, tag="m2", name=f"u2{h0}_{k}")
                    u3 = wpp.tile([P, TC2], F32, tag="u3", name=f"u3{h0}_{k}")
                    u4 = wpp.tile([P, TC2], F32, tag="u4", name=f"u4{h0}_{k}")
                    ts_mul(u1[:, :cwn], pa[:, :cwn], 9 + k)
                    ts_mul(u2[:, :cwn], pb[:, :cwn], 12 + k)
                    ts_mul(u3[:, :cwn], qa[:, :cwn], 15 + k)
                    ts_mul(u4[:, :cwn], qb[:, :cwn], 18 + k)
                    nc.vector.tensor_add(u1[:, :cwn], u1[:, :cwn],
                                         u2[:, :cwn])
                    nc.vector.tensor_add(u3[:, :cwn], u3[:, :cwn],
                                         u4[:, :cwn])
                    nc.vector.tensor_add(u1[:, :cwn], u1[:, :cwn],
                                         u3[:, :cwn])
                    nc.scalar.activation(wk[k][:, cw0:cw0 + cwn],
                                         u1[:, :cwn],
                                         mybir.ActivationFunctionType.Exp,
                                         bias=spar[:, 6 + k:7 + k])
                    nc.scalar.copy(
                        wk2[k][:, 2 * cw0:2 * (cw0 + cwn)].rearrange(
                            "p (c t) -> p c t", t=2),
                        wk[k][:, cw0:cw0 + cwn].rearrange(
                            "p (c u) -> p c u", u=1).to_broadcast(
                            (P, cwn, 2)))

            table_pass(HT, NT, hp_hi, HALF, xblkp, psA,
                       act_only=copies_act)
            psA_cm.__exit__(None, None, None)
            xblkp_cm.__exit__(None, None, None)

            # ---- phase B: edge loop (lo pass, then hi) ----
            h_sbuf = cst.tile([OUT, TP], BF16)
            gat_tiles = {}

            def issue_batch(b):
                if b in gat_tiles:
                    return
                if b < nb0:
                    c0 = b * G
                    gn = min(G, NSEG0 - c0)
                    src_tbl = hp_lo[:, :]
                else:
                    c0 = NSEG0 + (b - nb0) * G
                    gn = min(G, TC - c0)
                    src_tbl = hp_hi[:, :]
                gat = gatp.tile([P, G * ROWP], BF16, tag="gat",
                                name=f"gat{b}")
                gv = gat[:].rearrange("p (j f) -> p j f", f=ROWP)[:, :gn, :]
                nc.gpsimd.dma_gather(
                    out_ap=gv, in_ap=src_tbl,
                    idxs_ap=gat_s[:, b * SB:b * SB + gn * P // 16],
                    num_idxs=gn * P, num_idxs_reg=gn * P,
                    elem_size=ROWP, single_packet=False,
                    queue_num=b % 4)
                gat3 = gat[:].rearrange("p (j f) -> p j f", f=ROWP)
                mks = []
                for k in range(K):
                    mk = msgp.tile([P, G * OUT], BF16, tag=f"mk{k}",
                                   name=f"mk{k}_{b}")
                    nc.vector.tensor_tensor(
                        out=mk[:].rearrange("p (j m t) -> p j m t",
                                            m=OUT // 2, t=2)[:, :gn],
                        in0=gat3[:, :gn, k * OUT:(k + 1) * OUT].rearrange(
                            "p j (m t) -> p j m t", t=2),
                        in1=wk2[k][:, 2 * c0:2 * (c0 + gn)].rearrange(
                            "p (c u t) -> p c u t", u=1, t=2).to_broadcast(
                            (P, gn, OUT // 2, 2)),
                        op=mybir.AluOpType.mult)
                    mks.append(mk)
                msg = msgp.tile([P, G * OUT], BF16, tag="msg",
                                name=f"msg{b}")
                nc.vector.tensor_add(msg[:, :gn * OUT], mks[0][:, :gn * OUT],
                                     mks[1][:, :gn * OUT])
                nc.vector.tensor_add(msg[:, :gn * OUT], msg[:, :gn * OUT],
                                     mks[2][:, :gn * OUT])
                if (b % 8) < oh_host_per8:
                    oh = ohp.tile([P, G * P], FP8, tag="oh", name=f"oh{b}")
                    nc.scalar.dma_start(out=oh[:, :gn * P],
                                        in_=oh_d[:, c0 * P:(c0 + gn) * P])
                else:
                    oh = ohp.tile([P, G * P], BF16, tag="ohb",
                                  name=f"oh{b}")
                    nc.vector.tensor_tensor(
                        out=oh[:].rearrange("p (j m t) -> p j m t",
                                            m=P // 2, t=2)[:, :gn],
                        in0=dl_b[:, 2 * c0:2 * (c0 + gn)].rearrange(
                            "p (c u t) -> p c u t", u=1, t=2).to_broadcast(
                            (P, gn, P // 2, 2)),
                        in1=iota_f[:].rearrange(
                            "p (u m t) -> p u m t", u=1, t=2).to_broadcast(
                            (P, gn, P // 2, 2)),
                        op=mybir.AluOpType.is_equal)
                gat_tiles[b] = (oh, msg)

            def seg_pass(seg):
                Cseg = C1 if seg else C0
                seg0 = NSEG0 if seg else 0
                b0 = nb0 if seg else 0
                for t in range(T):
                    cur_ps = psB.tile([OUT, P], F32, tag="acc",
                                      name=f"acc{seg}_{t}")
                    for ci in range(Cseg):
                        col = seg0 + t * Cseg + ci
                        b = b0 + (col - seg0) // G
                        s = (col - seg0) % G
                        issue_batch(b)
                        oh, msg = gat_tiles[b]
                        nc.tensor.matmul(
                            cur_ps[:],
                            lhsT=msg[:, s * OUT:(s + 1) * OUT],
                            rhs=oh[:, s * P:(s + 1) * P],
                            start=(ci == 0), stop=(ci == Cseg - 1))
                    if not seg:
                        nc.scalar.activation(
                            h_sbuf[:, t * P:(t + 1) * P], cur_ps[:],
                            mybir.ActivationFunctionType.Identity,
                            bias=sbias[:, 0:1])
                    else:
                        nc.vector.tensor_add(
                            h_sbuf[:, t * P:(t + 1) * P],
                            h_sbuf[:, t * P:(t + 1) * P], cur_ps[:])

            seg_pass(0)
            seg_pass(1)
            NW = 4
            for wq in range(NW):
                ta = (T * wq) // NW
                tb = (T * (wq + 1)) // NW
                nc.sync.dma_start(out=hout_d[:, ta * P:tb * P],
                                  in_=h_sbuf[:, ta * P:tb * P])

    nc.compile()
    return nc


# ----------------------------------------------------------------------------
# Full model runner
# ----------------------------------------------------------------------------

_KERNEL_CACHE = {}


def _get_kernel(key, builder):
    if key not in _KERNEL_CACHE:
        _KERNEL_CACHE[key] = builder()
    return _KERNEL_CACHE[key]


def run_monet(inputs, n_cores=8, G=16, trace=False, OH0=8, OH1=2):
    feat = np.asarray(inputs["feat"], np.float32)
    pseudo = np.asarray(inputs["pseudo"], np.float32)
    src = np.asarray(inputs["src"], np.int32)
    dst = np.asarray(inputs["dst"], np.int32)
    N, IN_F = feat.shape
    HID = np.asarray(inputs["fc0"]).shape[0] // 3
    OUTF = np.asarray(inputs["fc1"]).shape[0] // 3
    K = 3

    md = build_edge_metadata(src, dst, pseudo, N, n_cores, G)
    T, NT = md["T"], md["NT"]
    NPAD = NT * P
    TP = T * P

    bfdt = BFDT
    featT = np.zeros((IN_F, NPAD), np.float32)
    featT[:, :N] = feat.T
    featT = np.ascontiguousarray(featT.astype(bfdt))
    ROWP0 = -(-(K * HID) // 128) * 128
    fc0T = np.zeros((IN_F, ROWP0), np.float32)
    fc0T[:, :K * HID] = np.asarray(inputs["fc0"], np.float32).T
    fc0T = np.ascontiguousarray(fc0T.astype(bfdt))
    ROWP1 = -(-(K * OUTF) // 128) * 128
    fc1T = np.zeros((HID, ROWP1), np.float32)
    fc1T[:, :K * OUTF] = np.asarray(inputs["fc1"], np.float32).T
    fc1T = np.ascontiguousarray(fc1T.astype(bfdt))
    par0 = pack_params(np.asarray(inputs["pp0_w"], np.float32),
                       np.asarray(inputs["pp0_b"], np.float32),
                       np.asarray(inputs["mu0"], np.float32),
                       np.asarray(inputs["inv_sigma0"], np.float32))
    par1 = pack_params(np.asarray(inputs["pp1_w"], np.float32),
                       np.asarray(inputs["pp1_b"], np.float32),
                       np.asarray(inputs["mu1"], np.float32),
                       np.asarray(inputs["inv_sigma1"], np.float32))
    b0 = np.asarray(inputs["b0"], np.float32).reshape(HID, 1)
    b1 = np.asarray(inputs["b1"], np.float32).reshape(OUTF, 1)

    nc0 = _get_kernel(("l0v5", NT, T, md["C0"], md["C1"], IN_F, HID, G,
                       n_cores, OH0),
                      lambda: build_layer_kernel(NT, T, md["C0"], md["C1"],
                                                 IN_F, HID, K, G, n_cores,
                                                 oh_host_per8=OH0))
    in_maps0 = []
    for c in range(n_cores):
        in_maps0.append(dict(
            xT=featT, wT=fc0T,
            gat_w=md["gat_w"][c], dl_t=md["dl_bf"][c], oh_w=md["oh_w"][c],
            ps_a=md["psa_bf"][c], ps_b=md["psb_bf"][c],
            par=par0, bias=b0))
    res0 = bass_utils.run_bass_kernel_spmd(
        nc0, in_maps0, core_ids=list(range(n_cores)), trace=trace)

    # h_out is (HID, TP) per core -> concat columns = h^T
    hcat = np.concatenate(
        [np.asarray(res0.results[c]["h_out"], np.float32)
         for c in range(n_cores)], axis=1)
    hT = np.zeros((HID, NPAD), np.float32)
    hT[:, :N] = hcat[:, :N]
    hT = np.ascontiguousarray(hT.astype(bfdt))

    nc1 = _get_kernel(("l1v5", NT, T, md["C0"], md["C1"], HID, OUTF, G,
                       n_cores, OH1),
                      lambda: build_layer_kernel(NT, T, md["C0"], md["C1"],
                                                 HID, OUTF, K, G, n_cores,
                                                 oh_host_per8=OH1))
    in_maps1 = []
    for c in range(n_cores):
        in_maps1.append(dict(
            xT=hT, wT=fc1T,
            gat_w=md["gat_w"][c], dl_t=md["dl_bf"][c], oh_w=md["oh_w"][c],
            ps_a=md["psa_bf"][c], ps_b=md["psb_bf"][c],
            par=par1, bias=b1))
    res1 = bass_utils.run_bass_kernel_spmd(
        nc1, in_maps1, core_ids=list(range(n_cores)), trace=trace)

    ocat = np.concatenate(
        [np.asarray(res1.results[c]["h_out"], np.float32)
         for c in range(n_cores)], axis=1)
    out = np.ascontiguousarray(ocat[:, :N].T)
    perf = dict(l0_ns=res0.exec_time_ns, l1_ns=res1.exec_time_ns)
    return out, perf


# ----------------------------------------------------------------------------
# Harness entry: full inputs in, full output out
# ----------------------------------------------------------------------------

def kernel(**inputs):
    out, _ = run_monet(inputs)
    return out.astype(np.float32)
